# revision 5
# baseline (speedup 1.0000x reference)
"""Trainium2 Bass kernel for nn_Extractor (back-project + trilinear volume sampling).

Strategy (8 NeuronCores, data-parallel over pixels):
  - Shard the h*w pixel dimension across 8 cores; each core gets 128x300 pixels.
  - Host precomputes a "halo-block" table H[cell] = the 8 corner values
    V[x..x+1, y..y+1, z..z+1] stored contiguously (32B per cell).  One
    indirect-DMA descriptor then fetches all 8 corners of a ray point.
  - On device per chunk of pixels: back-project to voxel coords, floor/frac,
    trilinear weights with out-of-bounds masking folded in, idx expansion,
    indirect gathers from H, weighted corner reduction.
"""
import os
import numpy as np

import concourse.bass as bass
import concourse.bacc as bacc
import concourse.mybir as mybir
from concourse.tile import TileContext
from concourse import bass_utils

F32 = mybir.dt.float32
I32 = mybir.dt.int32
Alu = mybir.AluOpType
P = 128

_NC_CACHE = {}
LAST_EXEC_NS = None  # set when KERNEL_TRACE=1
LAST_RES = None
LAST_H = None


def _build(cc, ch, v3):
    """Build the per-core Bass kernel.

    cc: pixel columns per partition (pixels per core = 128*cc)
    ch: chunk size in pixel columns (must divide cc)
    v3: volume side length (table has v3^3 cells, 8 f32 each)
    """
    assert cc % ch == 0
    nch = cc // ch
    lv = int(np.log2(v3))
    assert 1 << lv == v3
    hsize = ((v3 + 1) ** 3) * 8

    nc = bacc.Bacc()
    d_depth = nc.dram_tensor("depth", [P, cc], F32, kind="ExternalInput")
    d_r0 = [nc.dram_tensor(f"r0{d}", [P, cc], F32, kind="ExternalInput") for d in "xyz"]
    d_halo = nc.dram_tensor("halo", [1, hsize], F32, kind="ExternalInput")
    d_consts = nc.dram_tensor("consts", [P, 3], F32, kind="ExternalInput")
    d_offs = nc.dram_tensor("offs", [P, 9], F32, kind="ExternalInput")
    d_ct = [nc.dram_tensor(f"ct{d}", [P, 8], I32, kind="ExternalInput") for d in range(3)]

    o_int = nc.dram_tensor("interp", [P, cc * 9], F32, kind="ExternalOutput")
    o_rp = nc.dram_tensor("rp", [P, cc * 27], F32, kind="ExternalOutput")
    o_dir = nc.dram_tensor("dirt", [P, cc * 3], F32, kind="ExternalOutput")
    o_idx = nc.dram_tensor("idx", [P, cc * 216], I32, kind="ExternalOutput")

    with TileContext(nc) as tc:
        with (
            tc.tile_pool(name="persist", bufs=1) as pp,
            tc.tile_pool(name="temps", bufs=1) as tp,
            tc.tile_pool(name="carry", bufs=2) as cp,
        ):
            # ---- persistent loads ----
            zt = pp.tile([P, cc], F32, tag="zt")
            nc.sync.dma_start(out=zt[:], in_=d_depth[:])
            r0 = []
            for i, d in enumerate("xyz"):
                t = pp.tile([P, cc], F32, tag=f"r0{d}")
                nc.sync.dma_start(out=t[:], in_=d_r0[i][:])
                r0.append(t)
            consts = pp.tile([P, 3], F32, tag="consts")
            nc.sync.dma_start(out=consts[:], in_=d_consts[:])
            offs = pp.tile([P, 9], F32, tag="offs")
            nc.sync.dma_start(out=offs[:], in_=d_offs[:])
            ct = []
            for d in range(3):
                t = pp.tile([P, 8], I32, tag=f"ct{d}")
                nc.sync.dma_start(out=t[:], in_=d_ct[d][:])
                ct.append(t)

            # ---- whole-core accumulators (DMA'd out once) ----
            intac = pp.tile([P, cc * 9], F32, tag="intac")
            rpac = pp.tile([P, cc * 27], F32, tag="rpac")
            dirac = pp.tile([P, cc * 3], F32, tag="dirac")

            rv = rpac[:].rearrange("p (x j d) -> p x j d", j=9, d=3)
            dv = dirac[:].rearrange("p (x d) -> p x d", d=3)

            KD = [consts[:, i : i + 1] for i in range(3)]

            n9 = 9 * ch
            for ci in range(nch):
                s = ci * ch
                sfx = ""  # shared tags across chunks

                zc = zt[:, s : s + ch]
                # ---- per-pixel stage ----
                c3 = []
                for d in range(3):
                    t = tp.tile([P, ch], F32, tag=f"c{d}{sfx}")
                    nc.vector.tensor_tensor(out=t[:], in0=zc, in1=r0[d][:, s : s + ch], op=Alu.mult)
                    c3.append(t)
                n2 = tp.tile([P, ch], F32, tag=f"n2{sfx}")
                sq = tp.tile([P, ch], F32, tag=f"sq{sfx}")
                nc.vector.tensor_tensor(out=n2[:], in0=c3[0][:], in1=c3[0][:], op=Alu.mult)
                nc.vector.tensor_tensor(out=sq[:], in0=c3[1][:], in1=c3[1][:], op=Alu.mult)
                n2b = tp.tile([P, ch], F32, tag=f"n2b{sfx}")
                nc.vector.tensor_tensor(out=n2b[:], in0=n2[:], in1=sq[:], op=Alu.add)
                nc.vector.tensor_tensor(out=sq[:], in0=c3[2][:], in1=c3[2][:], op=Alu.mult)
                nc.vector.tensor_tensor(out=n2[:], in0=n2b[:], in1=sq[:], op=Alu.add)
                nr = tp.tile([P, ch], F32, tag=f"nr{sfx}")
                nc.scalar.sqrt(nr[:], n2[:])
                inv = tp.tile([P, ch], F32, tag=f"inv{sfx}")
                nc.vector.reciprocal(inv[:], nr[:])
                dir3 = []
                for d in range(3):
                    t = tp.tile([P, ch], F32, tag=f"dir{d}{sfx}")
                    nc.vector.tensor_tensor(out=t[:], in0=c3[d][:], in1=inv[:], op=Alu.mult)
                    dir3.append(t)
                    nc.scalar.copy(out=dv[:, s : s + ch, d], in_=t[:])
                cv3 = []
                for d in range(3):
                    t = tp.tile([P, ch], F32, tag=f"cv{d}{sfx}")
                    nc.vector.tensor_scalar(out=t[:], in0=c3[d][:], scalar1=KD[d], scalar2=None, op0=Alu.add)
                    cv3.append(t)

                # ---- ray points: rp_d[n = pix*9 + j] = cv_d[pix] + offs[j]*dir_d[pix] ----
                rp3 = []
                for d in range(3):
                    tmp = tp.tile([P, n9], F32, tag=f"rptmp{sfx}")
                    dir_b = dir3[d][:].unsqueeze(2).to_broadcast([P, ch, 9])
                    offs_b = offs[:].unsqueeze(1).to_broadcast([P, ch, 9])
                    tv = tmp[:].rearrange("p (x j) -> p x j", j=9)
                    nc.vector.tensor_tensor(out=tv, in0=dir_b, in1=offs_b, op=Alu.mult)
                    t = tp.tile([P, n9], F32, tag=f"rp{d}{sfx}")
                    cv_b = cv3[d][:].unsqueeze(2).to_broadcast([P, ch, 9])
                    nc.vector.tensor_tensor(out=t[:].rearrange("p (x j) -> p x j", j=9), in0=tv, in1=cv_b, op=Alu.add)
                    rp3.append(t)
                    # interleaved ray_points output (pix, j, d)
                    nc.scalar.copy(
                        out=rv[:, s : s + ch, :, d],
                        in_=t[:].rearrange("p (x j) -> p x j", j=9),
                    )

                # ---- floor + int base ----
                fl3, ib3 = [], []
                for d in range(3):
                    ti = tp.tile([P, n9], I32, tag=f"ti{sfx}")
                    nc.vector.tensor_copy(out=ti[:], in_=rp3[d][:])
                    tf = tp.tile([P, n9], F32, tag=f"tf{sfx}")
                    nc.vector.tensor_copy(out=tf[:], in_=ti[:])
                    gt = tp.tile([P, n9], F32, tag=f"gt{sfx}")
                    nc.vector.tensor_tensor(out=gt[:], in0=tf[:], in1=rp3[d][:], op=Alu.is_gt)
                    fl = tp.tile([P, n9], F32, tag=f"fl{d}{sfx}")
                    nc.vector.tensor_tensor(out=fl[:], in0=tf[:], in1=gt[:], op=Alu.subtract)
                    fl3.append(fl)
                    ib = tp.tile([P, n9], I32, tag=f"ib{d}{sfx}")
                    nc.vector.tensor_copy(out=ib[:], in_=fl[:])
                    ib3.append(ib)

                # ---- fractions and masked weight pairs ----
                G3 = []
                for d in range(3):
                    fr = tp.tile([P, n9], F32, tag=f"fr{sfx}")
                    nc.vector.tensor_tensor(out=fr[:], in0=rp3[d][:], in1=fl3[d][:], op=Alu.subtract)
                    om = tp.tile([P, n9], F32, tag=f"om{sfx}")
                    nc.vector.tensor_scalar(out=om[:], in0=fr[:], scalar1=-1.0, scalar2=1.0, op0=Alu.mult, op1=Alu.add)
                    ge0 = tp.tile([P, n9], F32, tag=f"ge0{sfx}")
                    nc.vector.tensor_scalar(out=ge0[:], in0=fl3[d][:], scalar1=0.0, scalar2=None, op0=Alu.is_ge)
                    leH = tp.tile([P, n9], F32, tag=f"leH{sfx}")
                    nc.vector.tensor_scalar(out=leH[:], in0=fl3[d][:], scalar1=float(v3 - 1), scalar2=None, op0=Alu.is_le)
                    gem = tp.tile([P, n9], F32, tag=f"gem{sfx}")
                    nc.vector.tensor_scalar(out=gem[:], in0=fl3[d][:], scalar1=-1.0, scalar2=None, op0=Alu.is_ge)
                    leL = tp.tile([P, n9], F32, tag=f"leL{sfx}")
                    nc.vector.tensor_scalar(out=leL[:], in0=fl3[d][:], scalar1=float(v3 - 2), scalar2=None, op0=Alu.is_le)
                    v0 = tp.tile([P, n9], F32, tag=f"v0{sfx}")
                    nc.vector.tensor_tensor(out=v0[:], in0=ge0[:], in1=leH[:], op=Alu.mult)
                    v1 = tp.tile([P, n9], F32, tag=f"v1{sfx}")
                    nc.vector.tensor_tensor(out=v1[:], in0=gem[:], in1=leL[:], op=Alu.mult)
                    G = tp.tile([P, n9 * 2], F32, tag=f"G{d}{sfx}")
                    Gv = G[:].rearrange("p (n s) -> p n s", s=2)
                    nc.vector.tensor_tensor(out=Gv[:, :, 0], in0=om[:], in1=v0[:], op=Alu.mult)
                    nc.vector.tensor_tensor(out=Gv[:, :, 1], in0=fr[:], in1=v1[:], op=Alu.mult)
                    G3.append(G)

                # ---- weight expansion: wyz then W ----
                wyz = tp.tile([P, n9 * 4], F32, tag=f"wyz{sfx}")
                gy = G3[1][:].rearrange("p (n s) -> p n s", s=2).unsqueeze(3).to_broadcast([P, n9, 2, 2])
                gz = G3[2][:].rearrange("p (n s) -> p n s", s=2).unsqueeze(2).to_broadcast([P, n9, 2, 2])
                nc.vector.tensor_tensor(
                    out=wyz[:].rearrange("p (n y z) -> p n y z", y=2, z=2), in0=gy, in1=gz, op=Alu.mult
                )
                W = cp.tile([P, n9 * 8], F32, tag="W")
                gx = G3[0][:].rearrange("p (n s) -> p n s", s=2).unsqueeze(3).to_broadcast([P, n9, 2, 4])
                wyz_b = wyz[:].rearrange("p (n c) -> p n c", c=4).unsqueeze(2).to_broadcast([P, n9, 2, 4])
                nc.vector.tensor_tensor(
                    out=W[:].rearrange("p (n x c) -> p n x c", x=2, c=4), in0=gx, in1=wyz_b, op=Alu.mult
                )

                # ---- idx output: ibase + CORNERS, interleaved (pix, j, c, d) ----
                idxc = cp.tile([P, ch * 216], I32, tag="idxc")
                iv = idxc[:].rearrange("p (n c d) -> p n c d", c=8, d=3)
                for d in range(3):
                    ib_b = ib3[d][:].unsqueeze(2).to_broadcast([P, n9, 8])
                    ct_b = ct[d][:].unsqueeze(1).to_broadcast([P, n9, 8])
                    nc.vector.tensor_tensor(out=iv[:, :, :, d], in0=ib_b, in1=ct_b, op=Alu.add)
                nc.sync.dma_start(out=o_idx[:, s * 216 : (s + ch) * 216], in_=idxc[:])

                # ---- gather offsets: cell i_d = clamp(base_d, -1, v3-1) + 1; off = ix*S1 + iy*S2 + iz*8
                S2 = (v3 + 1) * 8
                S1 = (v3 + 1) * S2
                offp = None
                for d, sc in ((0, S1), (1, S2), (2, 8)):
                    bcl = tp.tile([P, n9], I32, tag=f"bcl{sfx}")
                    nc.vector.tensor_scalar(out=bcl[:], in0=ib3[d][:], scalar1=-1, scalar2=v3 - 1, op0=Alu.max, op1=Alu.min)
                    shf = tp.tile([P, n9], I32, tag=f"shf{d}{sfx}")
                    nc.vector.tensor_scalar(out=shf[:], in0=bcl[:], scalar1=sc, scalar2=sc, op0=Alu.mult, op1=Alu.add)
                    if offp is None:
                        offp = shf
                    else:
                        if d == 1:
                            t = tp.tile([P, n9], I32, tag=f"offa{sfx}")
                        else:
                            t = cp.tile([P, n9], I32, tag="off1")
                        nc.vector.tensor_tensor(out=t[:], in0=offp[:], in1=shf[:], op=Alu.add)
                        offp = t
                off1 = offp

                # ---- indirect gathers: one 32B halo block per ray point ----
                val8 = cp.tile([P, n9 * 8], F32, tag="val8")
                for n in range(n9):
                    nc.gpsimd.indirect_dma_start(
                        out=val8[:, n * 8 : (n + 1) * 8],
                        out_offset=None,
                        in_=d_halo[:],
                        in_offset=bass.IndirectOffsetOnAxis(ap=off1[:, n : n + 1], axis=1),
                    )

                # ---- weighted corner reduction ----
                prod = cp.tile([P, n9 * 8], F32, tag="prod")
                nc.vector.tensor_tensor(out=prod[:], in0=val8[:], in1=W[:], op=Alu.mult)
                nc.vector.tensor_reduce(
                    out=intac[:, s * 9 : (s + ch) * 9],
                    in_=prod[:].rearrange("p (n c) -> p n c", c=8),
                    axis=mybir.AxisListType.X,
                    op=Alu.add,
                )

            nc.sync.dma_start(out=o_int[:], in_=intac[:])
            nc.sync.dma_start(out=o_rp[:], in_=rpac[:])
            nc.sync.dma_start(out=o_dir[:], in_=dirac[:])

    nc.finalize()
    return nc


def _halo_table(vol, v3):
    """H[i, j, k, (a,b,c)] = Vp2[i+a, j+b, k+c], i = cell_x + 1 in [0, v3]; [(v3+1)^3 * 8] f32."""
    vp = np.pad(vol, 1, mode="edge")          # [v3+2]^3
    n = v3 + 1
    H = np.empty((n, n, n, 8), dtype=np.float32)
    k = 0
    for a in (0, 1):
        for b in (0, 1):
            for c in (0, 1):
                H[:, :, :, k] = vp[a : a + n, b : b + n, c : c + n]
                k += 1
    return H.reshape(1, -1)


def _run(depth, extrinsics, intrinsics, global_volume, origin, n_cores, cc, ch, trace=False):
    global LAST_EXEC_NS
    b, h, w = depth.shape
    assert b == 1
    npix = h * w
    npc = npix // n_cores
    assert npc == P * cc
    v3 = global_volume.shape[0]

    K3 = np.asarray(intrinsics, np.float64)[0]
    E = np.asarray(extrinsics, np.float64)[0]
    R, t = E[:3, :3], E[:3, 3]
    M = R @ np.linalg.inv(K3)
    org = np.asarray(origin, np.float64)
    KD = (t - org).astype(np.float32)

    pix = np.arange(npix, dtype=np.int64)
    u = (pix % w).astype(np.float64)
    v = (pix // w).astype(np.float64)
    ray0 = (M[:, 0][:, None] * u[None] + M[:, 1][:, None] * v[None] + M[:, 2][:, None]).astype(np.float32)  # [3, npix]

    H = np.ascontiguousarray(_halo_table(np.asarray(global_volume, np.float32), v3))

    consts = np.tile(KD[None, :], (P, 1)).astype(np.float32)
    offs = np.tile(np.arange(-4, 5, dtype=np.float32)[None, :], (P, 1))
    corners = np.array([[i, j, k] for i in (0, 1) for j in (0, 1) for k in (0, 1)], dtype=np.int32)  # [8,3]
    cts = [np.tile(corners[:, d][None, :], (P, 1)).astype(np.int32) for d in range(3)]

    dflat = np.asarray(depth, np.float32).reshape(-1)
    in_maps = []
    for k in range(n_cores):
        sl = slice(k * npc, (k + 1) * npc)
        in_maps.append({
            "depth": dflat[sl].reshape(P, cc),
            "r0x": ray0[0, sl].reshape(P, cc),
            "r0y": ray0[1, sl].reshape(P, cc),
            "r0z": ray0[2, sl].reshape(P, cc),
            "halo": H,
            "consts": consts,
            "offs": offs,
            "ct0": cts[0], "ct1": cts[1], "ct2": cts[2],
        })

    key = (cc, ch, v3)
    if key not in _NC_CACHE:
        _NC_CACHE[key] = _build(cc, ch, v3)
    nc = _NC_CACHE[key]

    res = bass_utils.run_bass_kernel_spmd(nc, in_maps, core_ids=list(range(n_cores)), trace=trace)
    global LAST_RES, LAST_H
    LAST_RES, LAST_H = res, H
    if trace:
        LAST_EXEC_NS = res.exec_time_ns

    interp = np.empty((npix, 9), np.float32)
    rp = np.empty((npix, 9, 3), np.float32)
    dirn = np.empty((npix, 3), np.float32)
    idx = np.empty((npix, 9, 8, 3), np.int32)
    for k in range(n_cores):
        sl = slice(k * npc, (k + 1) * npc)
        o = res.results[k]
        interp[sl] = o["interp"].reshape(npc, 9)
        rp[sl] = o["rp"].reshape(npc, 9, 3)
        dirn[sl] = o["dirt"].reshape(npc, 3)
        idx[sl] = o["idx"].reshape(npc, 9, 8, 3)

    return (
        interp.reshape(1, npix, 9),
        rp.reshape(1, npix, 9, 3),
        dirn.reshape(1, npix, 3),
        idx.reshape(1, npix, 9, 8, 3),
    )


def kernel(depth, extrinsics, intrinsics, global_volume, origin, resolution=None, **_ignored):
    trace = os.environ.get("KERNEL_TRACE", "0") == "1"
    return _run(
        np.asarray(depth), np.asarray(extrinsics), np.asarray(intrinsics),
        np.asarray(global_volume), np.asarray(origin),
        n_cores=8, cc=300, ch=30, trace=trace,
    )


# revision 7
# speedup vs baseline: 14.7876x; 14.7876x over previous
"""Trainium2 Bass kernel for nn_Extractor (back-project + trilinear volume sampling).

Strategy (8 NeuronCores, data-parallel over pixels):
  - Shard the h*w pixel dimension across 8 cores; each core gets 128x300 pixels.
  - Host precomputes a "halo-block" table H[cell] = the 8 corner values
    V[x..x+1, y..y+1, z..z+1] stored contiguously (32B per cell).  One
    indirect-DMA descriptor then fetches all 8 corners of a ray point.
  - On device per chunk of pixels: back-project to voxel coords, floor/frac,
    trilinear weights with out-of-bounds masking folded in, idx expansion,
    indirect gathers from H, weighted corner reduction.
"""
import os
import numpy as np

import concourse.bass as bass
import concourse.bacc as bacc
import concourse.mybir as mybir
from concourse.tile import TileContext
from concourse import bass_utils

F32 = mybir.dt.float32
I32 = mybir.dt.int32
Alu = mybir.AluOpType
P = 128

_NC_CACHE = {}
LAST_EXEC_NS = None  # set when KERNEL_TRACE=1
LAST_RES = None
LAST_H = None


def _build(cc, ch, v3):
    """Build the per-core Bass kernel.

    cc: pixel columns per partition (pixels per core = 128*cc)
    ch: chunk size in pixel columns (must divide cc)
    v3: volume side length (table has v3^3 cells, 8 f32 each)
    """
    assert cc % ch == 0
    nch = cc // ch
    lv = int(np.log2(v3))
    assert 1 << lv == v3
    hsize = ((v3 + 1) ** 3) * 8

    nc = bacc.Bacc()
    d_depth = nc.dram_tensor("depth", [P, cc], F32, kind="ExternalInput")
    d_r0 = [nc.dram_tensor(f"r0{d}", [P, cc], F32, kind="ExternalInput") for d in "xyz"]
    d_halo = nc.dram_tensor("halo", [1, hsize], F32, kind="ExternalInput")
    d_consts = nc.dram_tensor("consts", [P, 3], F32, kind="ExternalInput")
    d_offs = nc.dram_tensor("offs", [P, 9], F32, kind="ExternalInput")
    d_ct = [nc.dram_tensor(f"ct{d}", [P, 8], I32, kind="ExternalInput") for d in range(3)]

    o_int = nc.dram_tensor("interp", [P, cc * 9], F32, kind="ExternalOutput")
    o_rp = nc.dram_tensor("rp", [P, cc * 27], F32, kind="ExternalOutput")
    o_dir = nc.dram_tensor("dirt", [P, cc * 3], F32, kind="ExternalOutput")
    o_idx = nc.dram_tensor("idx", [P, cc * 216], I32, kind="ExternalOutput")

    with TileContext(nc) as tc:
        with (
            tc.tile_pool(name="persist", bufs=1) as pp,
            tc.tile_pool(name="temps", bufs=1) as tp,
            tc.tile_pool(name="carry", bufs=2) as cp,
        ):
            # ---- persistent loads ----
            zt = pp.tile([P, cc], F32, tag="zt")
            nc.sync.dma_start(out=zt[:], in_=d_depth[:])
            r0 = []
            for i, d in enumerate("xyz"):
                t = pp.tile([P, cc], F32, tag=f"r0{d}")
                nc.sync.dma_start(out=t[:], in_=d_r0[i][:])
                r0.append(t)
            consts = pp.tile([P, 3], F32, tag="consts")
            nc.sync.dma_start(out=consts[:], in_=d_consts[:])
            offs = pp.tile([P, 9], F32, tag="offs")
            nc.sync.dma_start(out=offs[:], in_=d_offs[:])
            ct = []
            for d in range(3):
                t = pp.tile([P, 8], I32, tag=f"ct{d}")
                nc.sync.dma_start(out=t[:], in_=d_ct[d][:])
                ct.append(t)

            # ---- whole-core accumulators (DMA'd out once) ----
            intac = pp.tile([P, cc * 9], F32, tag="intac")
            rpac = pp.tile([P, cc * 27], F32, tag="rpac")
            dirac = pp.tile([P, cc * 3], F32, tag="dirac")

            rv = rpac[:].rearrange("p (x j d) -> p x j d", j=9, d=3)
            dv = dirac[:].rearrange("p (x d) -> p x d", d=3)

            KD = [consts[:, i : i + 1] for i in range(3)]

            n9 = 9 * ch
            for ci in range(nch):
                s = ci * ch
                sfx = ""  # shared tags across chunks

                zc = zt[:, s : s + ch]
                # ---- per-pixel stage ----
                c3 = []
                for d in range(3):
                    t = tp.tile([P, ch], F32, tag=f"c{d}{sfx}")
                    nc.vector.tensor_tensor(out=t[:], in0=zc, in1=r0[d][:, s : s + ch], op=Alu.mult)
                    c3.append(t)
                n2 = tp.tile([P, ch], F32, tag=f"n2{sfx}")
                sq = tp.tile([P, ch], F32, tag=f"sq{sfx}")
                nc.vector.tensor_tensor(out=n2[:], in0=c3[0][:], in1=c3[0][:], op=Alu.mult)
                nc.vector.tensor_tensor(out=sq[:], in0=c3[1][:], in1=c3[1][:], op=Alu.mult)
                n2b = tp.tile([P, ch], F32, tag=f"n2b{sfx}")
                nc.vector.tensor_tensor(out=n2b[:], in0=n2[:], in1=sq[:], op=Alu.add)
                nc.vector.tensor_tensor(out=sq[:], in0=c3[2][:], in1=c3[2][:], op=Alu.mult)
                nc.vector.tensor_tensor(out=n2[:], in0=n2b[:], in1=sq[:], op=Alu.add)
                nr = tp.tile([P, ch], F32, tag=f"nr{sfx}")
                nc.scalar.sqrt(nr[:], n2[:])
                inv = tp.tile([P, ch], F32, tag=f"inv{sfx}")
                nc.vector.reciprocal(inv[:], nr[:])
                dir3 = []
                for d in range(3):
                    t = tp.tile([P, ch], F32, tag=f"dir{d}{sfx}")
                    nc.vector.tensor_tensor(out=t[:], in0=c3[d][:], in1=inv[:], op=Alu.mult)
                    dir3.append(t)
                    nc.scalar.copy(out=dv[:, s : s + ch, d], in_=t[:])
                cv3 = []
                for d in range(3):
                    t = tp.tile([P, ch], F32, tag=f"cv{d}{sfx}")
                    nc.vector.tensor_scalar(out=t[:], in0=c3[d][:], scalar1=KD[d], scalar2=None, op0=Alu.add)
                    cv3.append(t)

                # ---- ray points: rp_d[n = pix*9 + j] = cv_d[pix] + offs[j]*dir_d[pix] ----
                rp3 = []
                for d in range(3):
                    tmp = tp.tile([P, n9], F32, tag=f"rptmp{sfx}")
                    dir_b = dir3[d][:].unsqueeze(2).to_broadcast([P, ch, 9])
                    offs_b = offs[:].unsqueeze(1).to_broadcast([P, ch, 9])
                    tv = tmp[:].rearrange("p (x j) -> p x j", j=9)
                    nc.vector.tensor_tensor(out=tv, in0=dir_b, in1=offs_b, op=Alu.mult)
                    t = tp.tile([P, n9], F32, tag=f"rp{d}{sfx}")
                    cv_b = cv3[d][:].unsqueeze(2).to_broadcast([P, ch, 9])
                    nc.vector.tensor_tensor(out=t[:].rearrange("p (x j) -> p x j", j=9), in0=tv, in1=cv_b, op=Alu.add)
                    rp3.append(t)
                    # interleaved ray_points output (pix, j, d)
                    nc.scalar.copy(
                        out=rv[:, s : s + ch, :, d],
                        in_=t[:].rearrange("p (x j) -> p x j", j=9),
                    )

                # ---- floor + int base ----
                fl3, ib3 = [], []
                for d in range(3):
                    ti = tp.tile([P, n9], I32, tag=f"ti{sfx}")
                    nc.vector.tensor_copy(out=ti[:], in_=rp3[d][:])
                    tf = tp.tile([P, n9], F32, tag=f"tf{sfx}")
                    nc.vector.tensor_copy(out=tf[:], in_=ti[:])
                    gt = tp.tile([P, n9], F32, tag=f"gt{sfx}")
                    nc.vector.tensor_tensor(out=gt[:], in0=tf[:], in1=rp3[d][:], op=Alu.is_gt)
                    fl = tp.tile([P, n9], F32, tag=f"fl{d}{sfx}")
                    nc.vector.tensor_tensor(out=fl[:], in0=tf[:], in1=gt[:], op=Alu.subtract)
                    fl3.append(fl)
                    ib = tp.tile([P, n9], I32, tag=f"ib{d}{sfx}")
                    nc.vector.tensor_copy(out=ib[:], in_=fl[:])
                    ib3.append(ib)

                # ---- fractions and masked weight pairs ----
                G3 = []
                for d in range(3):
                    fr = tp.tile([P, n9], F32, tag=f"fr{sfx}")
                    nc.vector.tensor_tensor(out=fr[:], in0=rp3[d][:], in1=fl3[d][:], op=Alu.subtract)
                    om = tp.tile([P, n9], F32, tag=f"om{sfx}")
                    nc.vector.tensor_scalar(out=om[:], in0=fr[:], scalar1=-1.0, scalar2=1.0, op0=Alu.mult, op1=Alu.add)
                    ge0 = tp.tile([P, n9], F32, tag=f"ge0{sfx}")
                    nc.vector.tensor_scalar(out=ge0[:], in0=fl3[d][:], scalar1=0.0, scalar2=None, op0=Alu.is_ge)
                    leH = tp.tile([P, n9], F32, tag=f"leH{sfx}")
                    nc.vector.tensor_scalar(out=leH[:], in0=fl3[d][:], scalar1=float(v3 - 1), scalar2=None, op0=Alu.is_le)
                    gem = tp.tile([P, n9], F32, tag=f"gem{sfx}")
                    nc.vector.tensor_scalar(out=gem[:], in0=fl3[d][:], scalar1=-1.0, scalar2=None, op0=Alu.is_ge)
                    leL = tp.tile([P, n9], F32, tag=f"leL{sfx}")
                    nc.vector.tensor_scalar(out=leL[:], in0=fl3[d][:], scalar1=float(v3 - 2), scalar2=None, op0=Alu.is_le)
                    v0 = tp.tile([P, n9], F32, tag=f"v0{sfx}")
                    nc.vector.tensor_tensor(out=v0[:], in0=ge0[:], in1=leH[:], op=Alu.mult)
                    v1 = tp.tile([P, n9], F32, tag=f"v1{sfx}")
                    nc.vector.tensor_tensor(out=v1[:], in0=gem[:], in1=leL[:], op=Alu.mult)
                    G = tp.tile([P, n9 * 2], F32, tag=f"G{d}{sfx}")
                    Gv = G[:].rearrange("p (n s) -> p n s", s=2)
                    nc.vector.tensor_tensor(out=Gv[:, :, 0], in0=om[:], in1=v0[:], op=Alu.mult)
                    nc.vector.tensor_tensor(out=Gv[:, :, 1], in0=fr[:], in1=v1[:], op=Alu.mult)
                    G3.append(G)

                # ---- weight expansion: wyz then W ----
                wyz = tp.tile([P, n9 * 4], F32, tag=f"wyz{sfx}")
                gy = G3[1][:].rearrange("p (n s) -> p n s", s=2).unsqueeze(3).to_broadcast([P, n9, 2, 2])
                gz = G3[2][:].rearrange("p (n s) -> p n s", s=2).unsqueeze(2).to_broadcast([P, n9, 2, 2])
                nc.vector.tensor_tensor(
                    out=wyz[:].rearrange("p (n y z) -> p n y z", y=2, z=2), in0=gy, in1=gz, op=Alu.mult
                )
                W = cp.tile([P, n9 * 8], F32, tag="W")
                gx = G3[0][:].rearrange("p (n s) -> p n s", s=2).unsqueeze(3).to_broadcast([P, n9, 2, 4])
                wyz_b = wyz[:].rearrange("p (n c) -> p n c", c=4).unsqueeze(2).to_broadcast([P, n9, 2, 4])
                nc.vector.tensor_tensor(
                    out=W[:].rearrange("p (n x c) -> p n x c", x=2, c=4), in0=gx, in1=wyz_b, op=Alu.mult
                )

                # ---- idx output: ibase + CORNERS, interleaved (pix, j, c, d) ----
                idxc = cp.tile([P, ch * 216], I32, tag="idxc")
                iv = idxc[:].rearrange("p (n c d) -> p n c d", c=8, d=3)
                for d in range(3):
                    ib_b = ib3[d][:].unsqueeze(2).to_broadcast([P, n9, 8])
                    ct_b = ct[d][:].unsqueeze(1).to_broadcast([P, n9, 8])
                    nc.vector.tensor_tensor(out=iv[:, :, :, d], in0=ib_b, in1=ct_b, op=Alu.add)
                nc.sync.dma_start(out=o_idx[:, s * 216 : (s + ch) * 216], in_=idxc[:])

                # ---- gather offsets: cell i_d = clamp(base_d, -1, v3-1) + 1; off = ix*S1 + iy*S2 + iz*8
                S2 = (v3 + 1) * 8
                S1 = (v3 + 1) * S2
                offp = None
                for d, sc in ((0, S1), (1, S2), (2, 8)):
                    bcl = tp.tile([P, n9], I32, tag=f"bcl{sfx}")
                    nc.vector.tensor_scalar(out=bcl[:], in0=ib3[d][:], scalar1=-1, scalar2=v3 - 1, op0=Alu.max, op1=Alu.min)
                    shf = tp.tile([P, n9], I32, tag=f"shf{d}{sfx}")
                    nc.vector.tensor_scalar(out=shf[:], in0=bcl[:], scalar1=sc, scalar2=sc, op0=Alu.mult, op1=Alu.add)
                    if offp is None:
                        offp = shf
                    else:
                        if d == 1:
                            t = tp.tile([P, n9], I32, tag=f"offa{sfx}")
                        else:
                            t = cp.tile([P, n9], I32, tag="off1")
                        nc.vector.tensor_tensor(out=t[:], in0=offp[:], in1=shf[:], op=Alu.add)
                        offp = t
                off1 = offp

                # ---- indirect gathers: one 32B halo block per ray point ----
                val8 = cp.tile([P, n9 * 8], F32, tag="val8")
                for n in range(n9):
                    nc.gpsimd.indirect_dma_start(
                        out=val8[:, n * 8 : (n + 1) * 8],
                        out_offset=None,
                        in_=d_halo[:],
                        in_offset=bass.IndirectOffsetOnAxis(ap=off1[:, n : n + 1], axis=1),
                    )

                # ---- weighted corner reduction ----
                prod = cp.tile([P, n9 * 8], F32, tag="prod")
                nc.vector.tensor_tensor(out=prod[:], in0=val8[:], in1=W[:], op=Alu.mult)
                nc.vector.tensor_reduce(
                    out=intac[:, s * 9 : (s + ch) * 9],
                    in_=prod[:].rearrange("p (n c) -> p n c", c=8),
                    axis=mybir.AxisListType.X,
                    op=Alu.add,
                )

            nc.sync.dma_start(out=o_int[:], in_=intac[:])
            nc.sync.dma_start(out=o_rp[:], in_=rpac[:])
            nc.sync.dma_start(out=o_dir[:], in_=dirac[:])

    nc.finalize()
    return nc



def _build_pair(cc, ch, v3):
    """Pair-brick variant: one 4x4x3-cell brick (48 f32) gathered per pair of
    consecutive ray points (j even leads j odd); odd points extract their 8
    corners from the brick via mask-selects.  Requires |dir_x|,|dir_y| <= 1
    and 0 <= dir_z <= 1 (floor shift per step in {-1,0,1} / {0,1})."""
    assert cc % ch == 0
    nch = cc // ch
    nb = v3 + 3
    S2 = nb * 48
    S1 = nb * S2
    hsize = (nb ** 3) * 48

    nc = bacc.Bacc()
    d_depth = nc.dram_tensor("depth", [P, cc], F32, kind="ExternalInput")
    d_r0 = [nc.dram_tensor(f"r0{d}", [P, cc], F32, kind="ExternalInput") for d in "xyz"]
    d_halo = nc.dram_tensor("halo", [1, hsize], F32, kind="ExternalInput")
    d_consts = nc.dram_tensor("consts", [P, 3], F32, kind="ExternalInput")
    d_offs = nc.dram_tensor("offs", [P, 9], F32, kind="ExternalInput")
    d_ct = [nc.dram_tensor(f"ct{d}", [P, 8], I32, kind="ExternalInput") for d in range(3)]

    o_int = nc.dram_tensor("interp", [P, cc * 9], F32, kind="ExternalOutput")
    o_rp = nc.dram_tensor("rp", [P, cc * 27], F32, kind="ExternalOutput")
    o_dir = nc.dram_tensor("dirt", [P, cc * 3], F32, kind="ExternalOutput")
    o_idx = nc.dram_tensor("idx", [P, cc * 216], I32, kind="ExternalOutput")

    with TileContext(nc) as tc:
        with (
            tc.tile_pool(name="persist", bufs=1) as pp,
            tc.tile_pool(name="temps", bufs=1) as tp,
            tc.tile_pool(name="carry", bufs=2) as cp,
        ):
            zt = pp.tile([P, cc], F32, tag="zt")
            nc.sync.dma_start(out=zt[:], in_=d_depth[:])
            r0 = []
            for i, d in enumerate("xyz"):
                t = pp.tile([P, cc], F32, tag=f"r0{d}")
                nc.sync.dma_start(out=t[:], in_=d_r0[i][:])
                r0.append(t)
            consts = pp.tile([P, 3], F32, tag="consts")
            nc.sync.dma_start(out=consts[:], in_=d_consts[:])
            offs = pp.tile([P, 9], F32, tag="offs")
            nc.sync.dma_start(out=offs[:], in_=d_offs[:])
            ct = []
            for d in range(3):
                t = pp.tile([P, 8], I32, tag=f"ct{d}")
                nc.sync.dma_start(out=t[:], in_=d_ct[d][:])
                ct.append(t)

            intac = pp.tile([P, cc * 9], F32, tag="intac")
            rpac = pp.tile([P, cc * 27], F32, tag="rpac")
            dirac = pp.tile([P, cc * 3], F32, tag="dirac")
            rv = rpac[:].rearrange("p (x j d) -> p x j d", j=9, d=3)
            dv = dirac[:].rearrange("p (x d) -> p x d", d=3)
            KD = [consts[:, i : i + 1] for i in range(3)]

            n9 = 9 * ch
            nl = 5 * ch
            no = 4 * ch
            for ci in range(nch):
                s = ci * ch
                zc = zt[:, s : s + ch]
                c3 = []
                for d in range(3):
                    t = tp.tile([P, ch], F32, tag=f"c{d}")
                    nc.vector.tensor_tensor(out=t[:], in0=zc, in1=r0[d][:, s : s + ch], op=Alu.mult)
                    c3.append(t)
                n2 = tp.tile([P, ch], F32, tag="n2")
                sq = tp.tile([P, ch], F32, tag="sq")
                nc.vector.tensor_tensor(out=n2[:], in0=c3[0][:], in1=c3[0][:], op=Alu.mult)
                nc.vector.tensor_tensor(out=sq[:], in0=c3[1][:], in1=c3[1][:], op=Alu.mult)
                n2b = tp.tile([P, ch], F32, tag="n2b")
                nc.vector.tensor_tensor(out=n2b[:], in0=n2[:], in1=sq[:], op=Alu.add)
                nc.vector.tensor_tensor(out=sq[:], in0=c3[2][:], in1=c3[2][:], op=Alu.mult)
                nc.vector.tensor_tensor(out=n2[:], in0=n2b[:], in1=sq[:], op=Alu.add)
                nr = tp.tile([P, ch], F32, tag="nr")
                nc.scalar.sqrt(nr[:], n2[:])
                inv = tp.tile([P, ch], F32, tag="inv")
                nc.vector.reciprocal(inv[:], nr[:])
                dir3 = []
                for d in range(3):
                    t = tp.tile([P, ch], F32, tag=f"dir{d}")
                    nc.vector.tensor_tensor(out=t[:], in0=c3[d][:], in1=inv[:], op=Alu.mult)
                    dir3.append(t)
                    nc.scalar.copy(out=dv[:, s : s + ch, d], in_=t[:])
                cv3 = []
                for d in range(3):
                    t = tp.tile([P, ch], F32, tag=f"cv{d}")
                    nc.vector.tensor_scalar(out=t[:], in0=c3[d][:], scalar1=KD[d], scalar2=None, op0=Alu.add)
                    cv3.append(t)

                rp3 = []
                for d in range(3):
                    tmp = tp.tile([P, n9], F32, tag="rptmp")
                    dir_b = dir3[d][:].unsqueeze(2).to_broadcast([P, ch, 9])
                    offs_b = offs[:].unsqueeze(1).to_broadcast([P, ch, 9])
                    tv = tmp[:].rearrange("p (x j) -> p x j", j=9)
                    nc.vector.tensor_tensor(out=tv, in0=dir_b, in1=offs_b, op=Alu.mult)
                    t = tp.tile([P, n9], F32, tag=f"rp{d}")
                    cv_b = cv3[d][:].unsqueeze(2).to_broadcast([P, ch, 9])
                    nc.vector.tensor_tensor(out=t[:].rearrange("p (x j) -> p x j", j=9), in0=tv, in1=cv_b, op=Alu.add)
                    rp3.append(t)
                    nc.scalar.copy(out=rv[:, s : s + ch, :, d], in_=t[:].rearrange("p (x j) -> p x j", j=9))

                fl3, ib3 = [], []
                for d in range(3):
                    ti = tp.tile([P, n9], I32, tag="ti")
                    nc.vector.tensor_copy(out=ti[:], in_=rp3[d][:])
                    tf = tp.tile([P, n9], F32, tag="tf")
                    nc.vector.tensor_copy(out=tf[:], in_=ti[:])
                    gt = tp.tile([P, n9], F32, tag="gt")
                    nc.vector.tensor_tensor(out=gt[:], in0=tf[:], in1=rp3[d][:], op=Alu.is_gt)
                    fl = tp.tile([P, n9], F32, tag=f"fl{d}")
                    nc.vector.tensor_tensor(out=fl[:], in0=tf[:], in1=gt[:], op=Alu.subtract)
                    fl3.append(fl)
                    ib = tp.tile([P, n9], I32, tag=f"ib{d}")
                    nc.vector.tensor_copy(out=ib[:], in_=fl[:])
                    ib3.append(ib)

                G3 = []
                for d in range(3):
                    fr = tp.tile([P, n9], F32, tag="fr")
                    nc.vector.tensor_tensor(out=fr[:], in0=rp3[d][:], in1=fl3[d][:], op=Alu.subtract)
                    om = tp.tile([P, n9], F32, tag="om")
                    nc.vector.tensor_scalar(out=om[:], in0=fr[:], scalar1=-1.0, scalar2=1.0, op0=Alu.mult, op1=Alu.add)
                    ge0 = tp.tile([P, n9], F32, tag="ge0")
                    nc.vector.tensor_scalar(out=ge0[:], in0=fl3[d][:], scalar1=0.0, scalar2=None, op0=Alu.is_ge)
                    leH = tp.tile([P, n9], F32, tag="leH")
                    nc.vector.tensor_scalar(out=leH[:], in0=fl3[d][:], scalar1=float(v3 - 1), scalar2=None, op0=Alu.is_le)
                    gem = tp.tile([P, n9], F32, tag="gem")
                    nc.vector.tensor_scalar(out=gem[:], in0=fl3[d][:], scalar1=-1.0, scalar2=None, op0=Alu.is_ge)
                    leL = tp.tile([P, n9], F32, tag="leL")
                    nc.vector.tensor_scalar(out=leL[:], in0=fl3[d][:], scalar1=float(v3 - 2), scalar2=None, op0=Alu.is_le)
                    v0 = tp.tile([P, n9], F32, tag="v0")
                    nc.vector.tensor_tensor(out=v0[:], in0=ge0[:], in1=leH[:], op=Alu.mult)
                    v1 = tp.tile([P, n9], F32, tag="v1")
                    nc.vector.tensor_tensor(out=v1[:], in0=gem[:], in1=leL[:], op=Alu.mult)
                    G = tp.tile([P, n9 * 2], F32, tag=f"G{d}")
                    Gv = G[:].rearrange("p (n s) -> p n s", s=2)
                    nc.vector.tensor_tensor(out=Gv[:, :, 0], in0=om[:], in1=v0[:], op=Alu.mult)
                    nc.vector.tensor_tensor(out=Gv[:, :, 1], in0=fr[:], in1=v1[:], op=Alu.mult)
                    G3.append(G)

                wyz = tp.tile([P, n9 * 4], F32, tag="wyz")
                gy = G3[1][:].rearrange("p (n s) -> p n s", s=2).unsqueeze(3).to_broadcast([P, n9, 2, 2])
                gz = G3[2][:].rearrange("p (n s) -> p n s", s=2).unsqueeze(2).to_broadcast([P, n9, 2, 2])
                nc.vector.tensor_tensor(out=wyz[:].rearrange("p (n y z) -> p n y z", y=2, z=2), in0=gy, in1=gz, op=Alu.mult)
                W = cp.tile([P, n9 * 8], F32, tag="W")
                gx = G3[0][:].rearrange("p (n s) -> p n s", s=2).unsqueeze(3).to_broadcast([P, n9, 2, 4])
                wyz_b = wyz[:].rearrange("p (n c) -> p n c", c=4).unsqueeze(2).to_broadcast([P, n9, 2, 4])
                nc.vector.tensor_tensor(out=W[:].rearrange("p (n x c) -> p n x c", x=2, c=4), in0=gx, in1=wyz_b, op=Alu.mult)

                idxc = cp.tile([P, ch * 216], I32, tag="idxc")
                iv = idxc[:].rearrange("p (n c d) -> p n c d", c=8, d=3)
                for d in range(3):
                    ib_b = ib3[d][:].unsqueeze(2).to_broadcast([P, n9, 8])
                    ct_b = ct[d][:].unsqueeze(1).to_broadcast([P, n9, 8])
                    nc.vector.tensor_tensor(out=iv[:, :, :, d], in0=ib_b, in1=ct_b, op=Alu.add)
                nc.sync.dma_start(out=o_idx[:, s * 216 : (s + ch) * 216], in_=idxc[:])

                # ---- pair-brick offsets from even-j leader bases ----
                offp = None
                for d, sc in ((0, S1), (1, S2), (2, 48)):
                    ibv = ib3[d][:].rearrange("p (x j) -> p x j", j=9)
                    bcl = tp.tile([P, nl], I32, tag="pbcl")
                    nc.vector.tensor_scalar(
                        out=bcl[:].rearrange("p (x l) -> p x l", l=5),
                        in0=ibv[:, :, 0:9:2], scalar1=-2, scalar2=v3, op0=Alu.max, op1=Alu.min)
                    shf = tp.tile([P, nl], I32, tag=f"pshf{d}")
                    nc.vector.tensor_scalar(out=shf[:], in0=bcl[:], scalar1=sc, scalar2=2 * sc, op0=Alu.mult, op1=Alu.add)
                    if offp is None:
                        offp = shf
                    else:
                        if d == 1:
                            t = tp.tile([P, nl], I32, tag="poffa")
                        else:
                            t = cp.tile([P, nl], I32, tag="off5")
                        nc.vector.tensor_tensor(out=t[:], in0=offp[:], in1=shf[:], op=Alu.add)
                        offp = t
                off5 = offp

                brick = cp.tile([P, nl * 48], F32, tag="brick")
                for m in range(nl):
                    nc.gpsimd.indirect_dma_start(
                        out=brick[:, m * 48 : (m + 1) * 48],
                        out_offset=None, in_=d_halo[:],
                        in_offset=bass.IndirectOffsetOnAxis(ap=off5[:, m : m + 1], axis=1))

                # ---- val8 assembly ----
                val8 = cp.tile([P, n9 * 8], F32, tag="val8")
                vV = val8[:].rearrange("p (x j c) -> p x j c", j=9, c=8)
                vB = brick[:].rearrange("p (x l q) -> p x l q", l=5, q=48)
                # even points: fixed positions (1+a, 1+b, 0..1)
                for a in (0, 1):
                    for b in (0, 1):
                        q0 = (1 + a) * 12 + (1 + b) * 3
                        nc.scalar.copy(out=vV[:, :, 0:9:2, 4 * a + 2 * b : 4 * a + 2 * b + 2],
                                       in_=vB[:, :, :, q0 : q0 + 2])
                # odd deltas and masks
                mx, my, mz = [], [], []
                for d in range(3):
                    dn = tp.tile([P, no], I32, tag="pdn")
                    ibv = ib3[d][:].rearrange("p (x j) -> p x j", j=9)
                    nc.vector.tensor_tensor(out=dn[:].rearrange("p (x o) -> p x o", o=4),
                                            in0=ibv[:, :, 1:9:2], in1=ibv[:, :, 0:8:2], op=Alu.subtract)
                    df = tp.tile([P, no], F32, tag="pdf")
                    nc.vector.tensor_copy(out=df[:], in_=dn[:])
                    if d < 2:
                        ms = []
                        for si in range(3):
                            m = tp.tile([P, no], F32, tag=f"pm{d}{si}")
                            nc.vector.tensor_scalar(out=m[:], in0=df[:], scalar1=float(si - 1), scalar2=None, op0=Alu.is_equal)
                            ms.append(m)
                        if d == 0:
                            mx = ms
                        else:
                            my = ms
                    else:
                        ms = []
                        for si in range(2):
                            m = tp.tile([P, no], F32, tag=f"pm2{si}")
                            nc.vector.tensor_scalar(out=m[:], in0=df[:], scalar1=float(si), scalar2=None, op0=Alu.is_equal)
                            ms.append(m)
                        mz = ms

                def mb(m, k):
                    return m[:].rearrange("p (x o) -> p x o", o=4).unsqueeze(3).to_broadcast([P, ch, 4, k])

                vBo = brick[:].rearrange("p (x l q) -> p x l q", l=5, q=48)
                x24 = tp.tile([P, no * 24], F32, tag="px24")
                vX = x24[:].rearrange("p (x o q) -> p x o q", o=4, q=24)
                ta = tp.tile([P, no * 12], F32, tag="pta")
                tb = tp.tile([P, no * 12], F32, tag="ptb")
                vta = ta[:].rearrange("p (x o q) -> p x o q", o=4, q=12)
                vtb = tb[:].rearrange("p (x o q) -> p x o q", o=4, q=12)
                for h in (0, 1):
                    nc.vector.tensor_tensor(out=vta, in0=mb(mx[0], 12), in1=vBo[:, :, 0:4, h * 12 : h * 12 + 12], op=Alu.mult)
                    nc.vector.tensor_tensor(out=vtb, in0=mb(mx[1], 12), in1=vBo[:, :, 0:4, (1 + h) * 12 : (1 + h) * 12 + 12], op=Alu.mult)
                    nc.vector.tensor_tensor(out=vta, in0=vta, in1=vtb, op=Alu.add)
                    nc.vector.tensor_tensor(out=vtb, in0=mb(mx[2], 12), in1=vBo[:, :, 0:4, (2 + h) * 12 : (2 + h) * 12 + 12], op=Alu.mult)
                    nc.vector.tensor_tensor(out=vX[:, :, :, h * 12 : h * 12 + 12], in0=vta, in1=vtb, op=Alu.add)

                y12 = tp.tile([P, no * 12], F32, tag="py12")
                vY = y12[:].rearrange("p (x o q) -> p x o q", o=4, q=12)
                tc6a = tp.tile([P, no * 6], F32, tag="ptc6a")
                tc6b = tp.tile([P, no * 6], F32, tag="ptc6b")
                v6a = tc6a[:].rearrange("p (x o q) -> p x o q", o=4, q=6)
                v6b = tc6b[:].rearrange("p (x o q) -> p x o q", o=4, q=6)
                for h in (0, 1):
                    base = h * 12
                    nc.vector.tensor_tensor(out=v6a, in0=mb(my[0], 6), in1=vX[:, :, :, base : base + 6], op=Alu.mult)
                    nc.vector.tensor_tensor(out=v6b, in0=mb(my[1], 6), in1=vX[:, :, :, base + 3 : base + 9], op=Alu.mult)
                    nc.vector.tensor_tensor(out=v6a, in0=v6a, in1=v6b, op=Alu.add)
                    nc.vector.tensor_tensor(out=v6b, in0=mb(my[2], 6), in1=vX[:, :, :, base + 6 : base + 12], op=Alu.mult)
                    nc.vector.tensor_tensor(out=vY[:, :, :, h * 6 : h * 6 + 6], in0=v6a, in1=v6b, op=Alu.add)

                t2a = tp.tile([P, no * 2], F32, tag="pt2a")
                t2b = tp.tile([P, no * 2], F32, tag="pt2b")
                v2a = t2a[:].rearrange("p (x o q) -> p x o q", o=4, q=2)
                v2b = t2b[:].rearrange("p (x o q) -> p x o q", o=4, q=2)
                for a in (0, 1):
                    for b in (0, 1):
                        base = a * 6 + 3 * b
                        nc.vector.tensor_tensor(out=v2a, in0=mb(mz[0], 2), in1=vY[:, :, :, base : base + 2], op=Alu.mult)
                        nc.vector.tensor_tensor(out=v2b, in0=mb(mz[1], 2), in1=vY[:, :, :, base + 1 : base + 3], op=Alu.mult)
                        cb = 4 * a + 2 * b
                        nc.vector.tensor_tensor(out=vV[:, :, 1:9:2, cb : cb + 2], in0=v2a, in1=v2b, op=Alu.add)

                prod = cp.tile([P, n9 * 8], F32, tag="prod")
                nc.vector.tensor_tensor(out=prod[:], in0=val8[:], in1=W[:], op=Alu.mult)
                nc.vector.tensor_reduce(
                    out=intac[:, s * 9 : (s + ch) * 9],
                    in_=prod[:].rearrange("p (n c) -> p n c", c=8),
                    axis=mybir.AxisListType.X, op=Alu.add)

            nc.sync.dma_start(out=o_int[:], in_=intac[:])
            nc.sync.dma_start(out=o_rp[:], in_=rpac[:])
            nc.sync.dma_start(out=o_dir[:], in_=dirac[:])

    nc.finalize()
    return nc


def _brick_table(vol, v3):
    """B[i,j,k,(gx,gy,gz)] = Vp[i+gx, j+gy, k+1+gz], Vp = pad(V, 3, edge); [(v3+3)^3 * 48] f32."""
    vp = np.pad(vol, 3, mode="edge")
    n = v3 + 3
    B = np.empty((n, n, n, 48), dtype=np.float32)
    q = 0
    for gx in range(4):
        for gy in range(4):
            for gz in range(3):
                B[:, :, :, q] = vp[gx : gx + n, gy : gy + n, 1 + gz : 1 + gz + n]
                q += 1
    return B.reshape(1, -1)


def _halo_table(vol, v3):
    """H[i, j, k, (a,b,c)] = Vp2[i+a, j+b, k+c], i = cell_x + 1 in [0, v3]; [(v3+1)^3 * 8] f32."""
    vp = np.pad(vol, 1, mode="edge")          # [v3+2]^3
    n = v3 + 1
    H = np.empty((n, n, n, 8), dtype=np.float32)
    k = 0
    for a in (0, 1):
        for b in (0, 1):
            for c in (0, 1):
                H[:, :, :, k] = vp[a : a + n, b : b + n, c : c + n]
                k += 1
    return H.reshape(1, -1)


def _run(depth, extrinsics, intrinsics, global_volume, origin, n_cores, cc, ch, trace=False, mode="auto"):
    global LAST_EXEC_NS
    b, h, w = depth.shape
    assert b == 1
    npix = h * w
    npc = npix // n_cores
    assert npc == P * cc
    v3 = global_volume.shape[0]

    K3 = np.asarray(intrinsics, np.float64)[0]
    E = np.asarray(extrinsics, np.float64)[0]
    R, t = E[:3, :3], E[:3, 3]
    M = R @ np.linalg.inv(K3)
    org = np.asarray(origin, np.float64)
    KD = (t - org).astype(np.float32)

    pix = np.arange(npix, dtype=np.int64)
    u = (pix % w).astype(np.float64)
    v = (pix // w).astype(np.float64)
    ray0 = (M[:, 0][:, None] * u[None] + M[:, 1][:, None] * v[None] + M[:, 2][:, None]).astype(np.float32)  # [3, npix]

    ray_n = np.sqrt((ray0.astype(np.float64) ** 2).sum(0))
    dirs = ray0.astype(np.float64) / ray_n
    pair_ok = (
        np.abs(dirs[0]).max() <= 1.0 and np.abs(dirs[1]).max() <= 1.0
        and dirs[2].min() >= 0.0 and dirs[2].max() <= 1.0
    )
    if mode == "auto":
        mode = "pair" if pair_ok else "halo"
    if mode == "pair":
        H = np.ascontiguousarray(_brick_table(np.asarray(global_volume, np.float32), v3))
    else:
        H = np.ascontiguousarray(_halo_table(np.asarray(global_volume, np.float32), v3))

    consts = np.tile(KD[None, :], (P, 1)).astype(np.float32)
    offs = np.tile(np.arange(-4, 5, dtype=np.float32)[None, :], (P, 1))
    corners = np.array([[i, j, k] for i in (0, 1) for j in (0, 1) for k in (0, 1)], dtype=np.int32)  # [8,3]
    cts = [np.tile(corners[:, d][None, :], (P, 1)).astype(np.int32) for d in range(3)]

    dflat = np.asarray(depth, np.float32).reshape(-1)
    in_maps = []
    for k in range(n_cores):
        sl = slice(k * npc, (k + 1) * npc)
        in_maps.append({
            "depth": dflat[sl].reshape(P, cc),
            "r0x": ray0[0, sl].reshape(P, cc),
            "r0y": ray0[1, sl].reshape(P, cc),
            "r0z": ray0[2, sl].reshape(P, cc),
            "halo": H,
            "consts": consts,
            "offs": offs,
            "ct0": cts[0], "ct1": cts[1], "ct2": cts[2],
        })

    key = (cc, ch, v3, mode)
    if key not in _NC_CACHE:
        _NC_CACHE[key] = _build_pair(cc, ch, v3) if mode == "pair" else _build(cc, ch, v3)
    nc = _NC_CACHE[key]

    res = bass_utils.run_bass_kernel_spmd(nc, in_maps, core_ids=list(range(n_cores)), trace=trace)
    global LAST_RES, LAST_H
    LAST_RES, LAST_H = res, H
    if trace:
        LAST_EXEC_NS = res.exec_time_ns

    interp = np.empty((npix, 9), np.float32)
    rp = np.empty((npix, 9, 3), np.float32)
    dirn = np.empty((npix, 3), np.float32)
    idx = np.empty((npix, 9, 8, 3), np.int32)
    for k in range(n_cores):
        sl = slice(k * npc, (k + 1) * npc)
        o = res.results[k]
        interp[sl] = o["interp"].reshape(npc, 9)
        rp[sl] = o["rp"].reshape(npc, 9, 3)
        dirn[sl] = o["dirt"].reshape(npc, 3)
        idx[sl] = o["idx"].reshape(npc, 9, 8, 3)

    return (
        interp.reshape(1, npix, 9),
        rp.reshape(1, npix, 9, 3),
        dirn.reshape(1, npix, 3),
        idx.reshape(1, npix, 9, 8, 3),
    )


def kernel(depth, extrinsics, intrinsics, global_volume, origin, resolution=None, **_ignored):
    trace = os.environ.get("KERNEL_TRACE", "0") == "1"
    return _run(
        np.asarray(depth), np.asarray(extrinsics), np.asarray(intrinsics),
        np.asarray(global_volume), np.asarray(origin),
        n_cores=8, cc=300, ch=20, trace=trace,
    )
